# revision 9
# baseline (speedup 1.0000x reference)
"""Trainium2 Bass kernel for nn_Decoder (teacher-forced GRU decoder w/ additive
attention + vocab projection + log_softmax).

Sharding: data-parallel over batch across 8 cores (4 batches/core). Each core
runs the full T=100 recurrence for its batch shard, then projects all its
hidden states onto the vocab (V=32000) in two bf16 passes (stats, then write),
with log-softmax folded in.

kernel(**inputs) takes FULL inputs (as produced by reference.setup_inputs())
and returns the FULL outputs (dec_outputs, h_final, atts).
"""
import warnings
warnings.filterwarnings("ignore")
import numpy as np

import concourse.bass as bass
import concourse.mybir as mybir
import concourse.tile as tile
from concourse import bacc
from concourse.masks import make_identity

F32 = mybir.dt.float32
BF16 = mybir.dt.bfloat16
I32 = mybir.dt.int32

B, S, H, V, TFULL = 32, 128, 256, 32000, 100
NCORES = 8
NB = B // NCORES          # batches per core = 4
VT = 512                  # vocab tile width
NVT = (V + VT - 1) // VT  # 63 (62x512 + 1x256)


def emit(tc, outs, ins, T):
    """Emit the per-core program. outs/ins are tuples of DRAM APs."""
    nc = tc.nc
    dec, hfin, att = outs
    (enc, pack, idx, tbl, qwt, kwt, vwt, wht, wit, wvt, obr) = ins

    R = T * NB            # hidden-state rows (excluding init)
    NCH = (R + 127) // 128  # row chunks for projection
    GA = 20 if T % 20 == 0 else T  # atts DMA group

    from contextlib import ExitStack
    with ExitStack() as ctx:
        cp = ctx.enter_context(tc.tile_pool(name="const", bufs=1))
        wp = ctx.enter_context(tc.tile_pool(name="work", bufs=2))

        # ---------------- preamble: constants ----------------
        ident = cp.tile([128, 128], F32)
        make_identity(nc, ident[:])
        onesc = cp.tile([128, 1], F32)
        nc.vector.memset(onesc[:], 1.0)
        onesr = cp.tile([1, 128], F32)
        nc.vector.memset(onesr[:], 1.0)
        ones_bf = cp.tile([1, 128], BF16)
        nc.vector.memset(ones_bf[:], 1.0)

        # weights
        QwT = cp.tile([128, 2, 256], F32)
        nc.sync.dma_start(QwT[:], qwt.rearrange("(k p) o -> p k o", p=128))
        KwT = cp.tile([128, 2, 256], F32)
        nc.sync.dma_start(KwT[:], kwt.rearrange("(k p) o -> p k o", p=128))
        VwT = cp.tile([128, 2, 1], F32)
        nc.sync.dma_start(VwT[:], vwt.rearrange("(k p) o -> p k o", p=128))
        WhhT = cp.tile([128, 2, 768], BF16)
        nc.gpsimd.dma_start(WhhT[:], wht.rearrange("(k p) o -> p k o", p=128))
        WieT = cp.tile([128, 2, 768], BF16)
        nc.gpsimd.dma_start(WieT[:], wit[0:256, :].rearrange("(k p) o -> p k o", p=128))
        WicT = cp.tile([128, 2, 768], BF16)
        nc.gpsimd.dma_start(WicT[:], wit[256:512, :].rearrange("(k p) o -> p k o", p=128))
        packT = cp.tile([128, 24], F32)
        nc.sync.dma_start(packT[:], pack[:])
        bihT = packT[:, 0:6]
        bhhT = packT[:, 6:12]
        ob_bf = cp.tile([1, V], BF16)
        nc.gpsimd.dma_start(ob_bf[:], obr[:])

        # bias combos
        kqb = cp.tile([128, 2], F32)
        nc.vector.tensor_tensor(out=kqb[:], in0=packT[:, 14:16], in1=packT[:, 12:14],
                                op=mybir.AluOpType.add)
        bcomb = cp.tile([128, 6], F32)
        nc.vector.tensor_tensor(out=bcomb[:], in0=bihT, in1=bhhT, op=mybir.AluOpType.add)
        nc.vector.tensor_copy(out=bcomb[:, 4:6], in_=bihT[:, 4:6])  # n-part: b_ih only

        # enc loads + transposes
        encT = cp.tile([128, 2, NB, 128], F32)   # [h_in-part, kh, b, s]
        encb = cp.tile([128, NB, 256], BF16)     # [s-part, b, h]
        pre_cm = tc.tile_pool(name="pre", bufs=2, space="PSUM")
        pre = pre_cm.__enter__()
        for b in range(NB):
            encN = wp.tile([128, 256], F32, tag="encN")
            nc.sync.dma_start(encN[:], enc[b])
            nc.gpsimd.dma_start(encb[:, b, :], enc[b])  # cast f32->bf16
            for kh in range(2):
                pt = pre.tile([128, 128], F32, tag="ptrans")
                nc.tensor.transpose(pt[:], encN[:, kh * 128:(kh + 1) * 128], ident[:])
                nc.vector.tensor_copy(out=encT[:, kh, b, :], in_=pt[:])

        # enc_k = Kw @ encT + (Kb + Qb)
        enck = cp.tile([128, 2, NB, 128], F32)   # [h_out-part, mt, b, s]
        for mt in range(2):
            pk = pre.tile([128, NB * 128], F32, tag="penck")
            for kt in range(2):
                nc.tensor.matmul(pk[:], KwT[:, kt, mt * 128:(mt + 1) * 128],
                                 encT[:, kt].rearrange("p b s -> p (b s)"),
                                 start=(kt == 0), stop=(kt == 1))
            nc.scalar.activation(out=enck[:, mt].rearrange("p b s -> p (b s)"), in_=pk[:],
                                 func=mybir.ActivationFunctionType.Identity,
                                 bias=kqb[:, mt:mt + 1], scale=1.0)

        # embedding gather + transpose -> embT bf16 [h-part, kt, r]
        embT = cp.tile([128, 2, R], BF16)
        nrow_ch = [min(128, R - 128 * c) for c in range(NCH)]
        for c in range(NCH):
            nr = nrow_ch[c]
            idxt = wp.tile([128, 1], I32, tag="idxt")
            nc.sync.dma_start(idxt[:nr], idx[128 * c:128 * c + nr])
            gt = wp.tile([128, 256], F32, tag="gath")
            nc.gpsimd.indirect_dma_start(
                out=gt[:nr], out_offset=None, in_=tbl[:],
                in_offset=bass.IndirectOffsetOnAxis(ap=idxt[:nr, 0:1], axis=0))
            for kh in range(2):
                pt = pre.tile([128, 128], F32, tag="ptrans")
                nc.tensor.transpose(pt[:, :nr], gt[:nr, kh * 128:(kh + 1) * 128], ident[:nr, :nr])
                nc.vector.tensor_copy(out=embT[:, kh, 128 * c:128 * c + nr], in_=pt[:, :nr])

        # gi_emb = W_ie @ emb + bias  -> [128, 6, R] f32
        giemb = cp.tile([128, 6, R], F32)
        for ot in range(6):
            pe_ = pre.tile([128, R], F32, tag="pgiemb")
            for kt in range(2):
                nc.tensor.matmul(pe_[:], WieT[:, kt, ot * 128:(ot + 1) * 128],
                                 embT[:, kt, :], start=(kt == 0), stop=(kt == 1))
            nc.scalar.activation(out=giemb[:, ot, :], in_=pe_[:],
                                 func=mybir.ActivationFunctionType.Identity,
                                 bias=bcomb[:, ot:ot + 1], scale=1.0)

        # hidden state history
        hT = cp.tile([128, 2, 4 * (T + 1)], F32)
        hbf = cp.tile([128, 2, 4 * (T + 1)], BF16)
        nc.vector.tensor_copy(out=hT[:, :, 0:4],
                              in_=packT[:, 16:24].rearrange("p (k b) -> p k b", k=2))
        nc.vector.tensor_copy(out=hbf[:, :, 0:4], in_=hT[:, :, 0:4])

        att_stage = cp.tile([NB, GA, 128], F32)

        pre_cm.__exit__(None, None, None)
        rec_cm1 = tc.tile_pool(name="ps", bufs=1, space="PSUM")
        rec_cm2 = tc.tile_pool(name="psg", bufs=2, space="PSUM")
        pp = rec_cm1.__enter__()
        pg_pool = rec_cm2.__enter__()

        # ---------------- recurrence ----------------
        for t in range(T):
            c0 = 4 * t        # input state cols
            c1 = 4 * (t + 1)  # output state cols
            rr = 4 * t        # row base for emb/gi_emb

            # gates psum: q 0:8 | gh 8:32 | gic 32:56
            pg = pg_pool.tile([128, 56], F32, tag="pg")
            for ot in range(2):
                for kt in range(2):
                    nc.tensor.matmul(pg[:, 4 * ot:4 * ot + 4],
                                     QwT[:, kt, ot * 128:(ot + 1) * 128],
                                     hT[:, kt, c0:c0 + 4],
                                     start=(kt == 0), stop=(kt == 1))
            for ot in range(6):
                for kt in range(2):
                    nc.tensor.matmul(pg[:, 8 + 4 * ot:12 + 4 * ot],
                                     WhhT[:, kt, ot * 128:(ot + 1) * 128],
                                     hbf[:, kt, c0:c0 + 4],
                                     start=(kt == 0), stop=(kt == 1))

            # attention: e = tanh(enc_k + q), scores, softmax, ctx
            e = wp.tile([128, 2, NB, 128], F32, tag="e")
            for kt in range(2):
                ep = wp.tile([128, NB, 128], F32, tag="ep")
                nc.vector.tensor_tensor(
                    out=ep[:], in0=enck[:, kt],
                    in1=pg[:, 4 * kt:4 * kt + 4].to_broadcast((128, 4, 128)),
                    op=mybir.AluOpType.add)
                nc.scalar.activation(out=e[:, kt].rearrange("p b s -> p (b s)"),
                                     in_=ep[:].rearrange("p b s -> p (b s)"),
                                     func=mybir.ActivationFunctionType.Tanh)
            ps_s = pp.tile([128, NB], F32, tag="ps_s")
            for b in range(NB):
                for kt in range(2):
                    nc.tensor.matmul(ps_s[:, b:b + 1], e[:, kt, b, :], VwT[:, kt, :],
                                     start=(kt == 0), stop=(kt == 1))
            u = wp.tile([128, NB], F32, tag="u")
            nc.scalar.activation(out=u[:], in_=ps_s[:],
                                 func=mybir.ActivationFunctionType.Exp)
            u_bf = wp.tile([128, NB], BF16, tag="u_bf")
            nc.vector.tensor_copy(out=u_bf[:], in_=u[:])

            # sums (parallel branch): rec_rep = 1/sum_s u  broadcast to all partitions
            ps_sum = pp.tile([1, NB], F32, tag="ps_sum")
            nc.tensor.matmul(ps_sum[:], onesc[:], u[:], start=True, stop=True)
            rec = wp.tile([1, NB], F32, tag="rec")
            nc.vector.reciprocal(out=rec[:], in_=ps_sum[:])
            ps_rec = pp.tile([128, NB], F32, tag="ps_rec")
            nc.tensor.matmul(ps_rec[:], onesr[:], rec[:], start=True, stop=True)

            # ctx (unnormalized) via PE, bf16
            pc = pp.tile([128, 2, NB], F32, tag="pc")
            for mt in range(2):
                for b in range(NB):
                    nc.tensor.matmul(pc[:, mt, b:b + 1],
                                     encb[:, b, mt * 128:(mt + 1) * 128],
                                     u_bf[:, b:b + 1], start=True, stop=True)
            cx_bf = wp.tile([128, 2, NB], BF16, tag="cx_bf")
            nc.vector.tensor_copy(out=cx_bf[:], in_=pc[:])

            # atts output: un = u * rec_rep, transpose, stage
            rec_sb = wp.tile([128, NB], F32, tag="rec_sb")
            nc.vector.tensor_copy(out=rec_sb[:], in_=ps_rec[:])
            un = wp.tile([128, NB], F32, tag="un")
            nc.vector.tensor_tensor(out=un[:], in0=u[:], in1=rec_sb[:],
                                    op=mybir.AluOpType.mult)
            ps_at = pp.tile([NB, 128], F32, tag="ps_at")
            nc.tensor.transpose(ps_at[:], un[:], ident[:])
            nc.vector.tensor_copy(out=att_stage[:, t % GA, :], in_=ps_at[:])
            if (t + 1) % GA == 0:
                t0 = t + 1 - GA
                nc.sync.dma_start(att[:, t0:t0 + GA, :], att_stage[:])

            # gi_ctx
            for ot in range(6):
                for kt in range(2):
                    nc.tensor.matmul(pg[:, 32 + 4 * ot:36 + 4 * ot],
                                     WicT[:, kt, ot * 128:(ot + 1) * 128],
                                     cx_bf[:, kt, :], start=(kt == 0), stop=(kt == 1))

            # gate elementwise
            gicn = wp.tile([128, 6, NB], F32, tag="gicn")
            rec_b = bass.AP(rec_sb[:].tensor, rec_sb[:].offset,
                            [rec_sb[:].ap[0], [0, 6], rec_sb[:].ap[1]])
            nc.vector.tensor_tensor(out=gicn[:], in0=pg[:, 32:56].rearrange(
                "p (o b) -> p o b", o=6), in1=rec_b, op=mybir.AluOpType.mult)
            rz1 = wp.tile([128, 4, NB], F32, tag="rz1")
            nc.vector.tensor_tensor(out=rz1[:], in0=pg[:, 8:24].rearrange(
                "p (o b) -> p o b", o=4), in1=gicn[:, 0:4, :], op=mybir.AluOpType.add)
            nc.vector.tensor_tensor(out=rz1[:], in0=rz1[:],
                                    in1=giemb[:, 0:4, rr:rr + 4],
                                    op=mybir.AluOpType.add)
            rzg = wp.tile([128, 4, NB], F32, tag="rzg")
            nc.scalar.activation(out=rzg[:].rearrange("p o b -> p (o b)"),
                                 in_=rz1[:].rearrange("p o b -> p (o b)"),
                                 func=mybir.ActivationFunctionType.Sigmoid)
            # n = tanh(gic_n + giemb_n + r*(gh_n + bhh_n))
            nh = wp.tile([128, 2, NB], F32, tag="nh")
            for j in range(2):
                nc.vector.scalar_tensor_tensor(
                    out=nh[:, j, :], in0=pg[:, 24 + 4 * j:28 + 4 * j],
                    scalar=bhhT[:, 4 + j:5 + j], in1=rzg[:, j, :],
                    op0=mybir.AluOpType.add, op1=mybir.AluOpType.mult)
            np1 = wp.tile([128, 2, NB], F32, tag="np1")
            nc.vector.tensor_tensor(out=np1[:], in0=gicn[:, 4:6, :],
                                    in1=giemb[:, 4:6, rr:rr + 4], op=mybir.AluOpType.add)
            nc.vector.tensor_tensor(out=np1[:], in0=np1[:], in1=nh[:],
                                    op=mybir.AluOpType.add)
            nt = wp.tile([128, 2, NB], F32, tag="nt")
            nc.scalar.activation(out=nt[:].rearrange("p k b -> p (k b)"),
                                 in_=np1[:].rearrange("p k b -> p (k b)"),
                                 func=mybir.ActivationFunctionType.Tanh)
            # h_new = n + z*(h - n)
            d = wp.tile([128, 2, NB], F32, tag="d")
            nc.vector.tensor_tensor(out=d[:], in0=hT[:, :, c0:c0 + 4], in1=nt[:],
                                    op=mybir.AluOpType.subtract)
            nc.vector.tensor_tensor(out=d[:], in0=d[:],
                                    in1=rzg[:, 2:4, :], op=mybir.AluOpType.mult)
            nc.vector.tensor_tensor(out=hT[:, :, c1:c1 + 4], in0=nt[:], in1=d[:],
                                    op=mybir.AluOpType.add)
            nc.vector.tensor_copy(out=hbf[:, :, c1:c1 + 4], in_=hT[:, :, c1:c1 + 4])

        if T % GA != 0:
            pass  # GA == T in that case, handled above

        # final hidden -> hfin (NB, 256)
        hf_sb = cp.tile([NB, 256], F32)
        for kt in range(2):
            pt = pp.tile([NB, 128], F32, tag="ps_at")
            nc.tensor.transpose(pt[:], hT[:, kt, 4 * T:4 * T + 4], ident[:])
            nc.vector.tensor_copy(out=hf_sb[:, kt * 128:(kt + 1) * 128], in_=pt[:])
        nc.sync.dma_start(hfin[:], hf_sb[:])
        rec_cm2.__exit__(None, None, None)
        rec_cm1.__exit__(None, None, None)

        # ---------------- projection ----------------
        sacc = cp.tile([128, NCH, NVT], F32)
        logz = cp.tile([128, NCH], F32)
        with tc.tile_pool(name="proj", bufs=3) as jp, \
             tc.tile_pool(name="projp", bufs=4, space="PSUM") as jpp:
            # pass 1: stats
            for vt in range(NVT):
                va = vt * VT
                nv = min(VT, V - va)
                wpan = jp.tile([128, 2, VT], BF16, tag="wpan")
                nc.gpsimd.dma_start(wpan[:, :, :nv],
                                    wvt[:, va:va + nv].rearrange("(k p) v -> p k v", p=128))
                for c in range(NCH):
                    nr = nrow_ch[c]
                    pj = jpp.tile([128, VT], F32, tag="pj")
                    nc.tensor.matmul(pj[:nr, :nv], ones_bf[:, :nr], ob_bf[:, va:va + nv],
                                     start=True, stop=False)
                    for kt in range(2):
                        nc.tensor.matmul(pj[:nr, :nv],
                                         hbf[:, kt, 4 + 128 * c:4 + 128 * c + nr],
                                         wpan[:, kt, :nv],
                                         start=False, stop=(kt == 1))
                    etmp = jp.tile([128, VT], F32, tag="etmp")
                    nc.scalar.activation(out=etmp[:nr, :nv], in_=pj[:nr, :nv],
                                         func=mybir.ActivationFunctionType.Exp,
                                         accum_out=sacc[:nr, c, vt:vt + 1])
            for c in range(NCH):
                nr = nrow_ch[c]
                ssum = jp.tile([128, 1], F32, tag="ssum")
                nc.vector.reduce_sum(out=ssum[:nr], in_=sacc[:nr, c, :],
                                     axis=mybir.AxisListType.X)
                nc.scalar.activation(out=logz[:nr, c:c + 1], in_=ssum[:nr],
                                     func=mybir.ActivationFunctionType.Ln)
            # pass 2: recompute + subtract + write
            for vt in range(NVT):
                va = vt * VT
                nv = min(VT, V - va)
                wpan = jp.tile([128, 2, VT], BF16, tag="wpan")
                nc.gpsimd.dma_start(wpan[:, :, :nv],
                                    wvt[:, va:va + nv].rearrange("(k p) v -> p k v", p=128))
                for c in range(NCH):
                    nr = nrow_ch[c]
                    tc_ = (128 * c) // 4
                    pj = jpp.tile([128, VT], F32, tag="pj")
                    nc.tensor.matmul(pj[:nr, :nv], ones_bf[:, :nr], ob_bf[:, va:va + nv],
                                     start=True, stop=False)
                    for kt in range(2):
                        nc.tensor.matmul(pj[:nr, :nv],
                                         hbf[:, kt, 4 + 128 * c:4 + 128 * c + nr],
                                         wpan[:, kt, :nv],
                                         start=False, stop=(kt == 1))
                    res = jp.tile([128, VT], F32, tag="res")
                    nc.vector.tensor_scalar(out=res[:nr, :nv], in0=pj[:nr, :nv],
                                            scalar1=logz[:nr, c:c + 1], scalar2=None,
                                            op0=mybir.AluOpType.subtract)
                    nc.sync.dma_start(
                        dec[:, tc_:tc_ + nr // 4, va:va + nv].rearrange("b t v -> t b v"),
                        res[:nr, :nv])


# ---------------- host-side driver ----------------

_CACHE = {}


def _build(T):
    key = T
    if key in _CACHE:
        return _CACHE[key]
    nc = bacc.Bacc("TRN2", debug=False, num_devices=NCORES)
    R = T * NB
    ins_spec = [
        ("enc", (NB, S, H), F32),
        ("pack", (128, 24), F32),
        ("idx", (R, 1), I32),
        ("tbl", (V, H), F32),
        ("qwt", (H, H), F32),
        ("kwt", (H, H), F32),
        ("vwt", (H, 1), F32),
        ("wht", (H, 3 * H), F32),
        ("wit", (2 * H, 3 * H), F32),
        ("wvt", (H, V), F32),
        ("obr", (1, V), F32),
    ]
    outs_spec = [
        ("dec", (NB, T, V), F32),
        ("hfin", (NB, H), F32),
        ("att", (NB, T, S), F32),
    ]
    ins = tuple(nc.dram_tensor(n, list(s), d, kind="ExternalInput").ap()
                for n, s, d in ins_spec)
    outs = tuple(nc.dram_tensor(n, list(s), d, kind="ExternalOutput").ap()
                 for n, s, d in outs_spec)
    with tile.TileContext(nc) as tc:
        emit(tc, outs, ins, T)
    nc.compile()
    _CACHE[key] = (nc, [n for n, _, _ in ins_spec], [n for n, _, _ in outs_spec])
    return _CACHE[key]


def _prep_inputs(inputs, T):
    """Build the 8 per-core input maps from full inputs (layout/shard only)."""
    enc_outputs = np.asarray(inputs["enc_outputs"], np.float32)
    enc_hidden = np.asarray(inputs["enc_hidden"], np.float32)
    target = np.asarray(inputs["target_tensor"])
    embedding = np.asarray(inputs["embedding"], np.float32)
    Qw = np.asarray(inputs["Qw"], np.float32); Qb = np.asarray(inputs["Qb"], np.float32)
    Kw = np.asarray(inputs["Kw"], np.float32); Kb = np.asarray(inputs["Kb"], np.float32)
    Vw = np.asarray(inputs["Vw"], np.float32)
    W_ih = np.asarray(inputs["W_ih"], np.float32)
    W_hh = np.asarray(inputs["W_hh"], np.float32)
    b_ih = np.asarray(inputs["b_ih"], np.float32)
    b_hh = np.asarray(inputs["b_hh"], np.float32)
    out_w = np.asarray(inputs["out_w"], np.float32)
    out_b = np.asarray(inputs["out_b"], np.float32)

    tokens = np.concatenate(
        [np.zeros((B, 1), target.dtype), target[:, :T - 1]], axis=1).astype(np.int32)

    shared = dict(
        tbl=np.ascontiguousarray(embedding),
        qwt=np.ascontiguousarray(Qw.T), kwt=np.ascontiguousarray(Kw.T),
        vwt=np.ascontiguousarray(Vw.T),
        wht=np.ascontiguousarray(W_hh.T), wit=np.ascontiguousarray(W_ih.T),
        wvt=np.ascontiguousarray(out_w.T),
        obr=np.ascontiguousarray(out_b.reshape(1, V)),
    )
    bias_part = np.concatenate([
        b_ih.reshape(6, 128).T, b_hh.reshape(6, 128).T,
        Qb.reshape(2, 128).T, Kb.reshape(2, 128).T], axis=1)  # (128, 16)
    in_maps = []
    for c in range(NCORES):
        bs = slice(NB * c, NB * (c + 1))
        idx = np.ascontiguousarray(
            tokens[bs, :T].T.reshape(T * NB, 1))  # r = t*NB + b
        h0t = enc_hidden[0, bs].T  # (256, 4)
        h0p = h0t.reshape(2, 128, NB).transpose(1, 0, 2).reshape(128, 2 * NB)
        m = dict(shared)
        m["enc"] = np.ascontiguousarray(enc_outputs[bs])
        m["pack"] = np.ascontiguousarray(
            np.concatenate([bias_part, h0p], axis=1).astype(np.float32))
        m["idx"] = idx
        in_maps.append(m)
    return in_maps


TRACE = False
LAST = {}


def kernel(**inputs):
    from concourse.bass_utils import run_bass_kernel_spmd
    T = TFULL
    nc, in_names, out_names = _build(T)
    in_maps = _prep_inputs(inputs, T)
    import time
    t0 = time.time()
    r = run_bass_kernel_spmd(nc, in_maps, core_ids=list(range(NCORES)), trace=TRACE)
    LAST["wall_s"] = time.time() - t0
    LAST["exec_time_ns"] = r.exec_time_ns
    LAST["result"] = r
    results = r.results
    dec = np.concatenate([results[c]["dec"] for c in range(NCORES)], axis=0)
    hfin = np.concatenate([results[c]["hfin"] for c in range(NCORES)], axis=0)[None]
    att = np.concatenate([results[c]["att"] for c in range(NCORES)], axis=0)
    return dec, hfin, att
